# revision 2
# baseline (speedup 1.0000x reference)
import sys

if "/opt/trn_rl_repo" not in sys.path:
    sys.path.insert(0, "/opt/trn_rl_repo")

import numpy as np

import concourse.bass as bass
import concourse.tile as tile
from concourse import bacc, mybir
from concourse.bass_utils import run_bass_kernel_spmd
from concourse.masks import make_identity, make_upper_triangular

F32 = mybir.dt.float32

# Problem shape (hardcoded per contract)
B, T, D = 4, 2048, 768
H, HD = 12, 64
N_CORES = 8
HEADS_PER_CORE = 6          # 12 heads / 2 groups
CPC = HEADS_PER_CORE * HD   # 384 qkv columns per core
TC = T // 128               # 16 token tiles of 128
DC = D // 128               # 6 chunks of the model dim
CC = CPC // 128             # 3 chunks of this core's head cols
OC = D // 128               # 6 output-col chunks
WT = 512                    # wide tile for big matmuls
TW = T // WT                # 4 wide token tiles

_CACHE = {}


def _build_nc():
    nc = bacc.Bacc("TRN2", target_bir_lowering=False, debug=False)

    xT = nc.dram_tensor("xT", [D, T], F32, kind="ExternalInput")
    wq = nc.dram_tensor("wq", [D, CPC], F32, kind="ExternalInput")
    wk = nc.dram_tensor("wk", [D, CPC], F32, kind="ExternalInput")
    wv = nc.dram_tensor("wv", [D, CPC], F32, kind="ExternalInput")
    bq = nc.dram_tensor("bq", [128, CC], F32, kind="ExternalInput")
    bk = nc.dram_tensor("bk", [128, CC], F32, kind="ExternalInput")
    bv = nc.dram_tensor("bv", [128, CPC], F32, kind="ExternalInput")
    wo = nc.dram_tensor("wo", [CPC, D], F32, kind="ExternalInput")
    yT = nc.dram_tensor("yT", [D, T], F32, kind="ExternalOutput")

    with tile.TileContext(nc) as tc:
        with tc.tile_pool(name="persist", bufs=1) as pp:
            # persistent activations
            qT_sb = pp.tile([128, CC, T], F32)     # q^T, head cols on partitions
            kT_sb = pp.tile([128, CC, T], F32)
            v_sb = pp.tile([128, TC, HEADS_PER_CORE, HD + 1], F32)  # v + ones col
            attn_sb = pp.tile([128, TC, CPC], F32)  # attention out [T, cols]
            wo_sb = pp.tile([128, CC, D], F32)
            bq_sb = pp.tile([128, CC], F32)
            bk_sb = pp.tile([128, CC], F32)
            bv_sb = pp.tile([128, CPC], F32)
            ident = pp.tile([128, 128], F32)
            mask01 = pp.tile([128, 128], F32)

            nc.sync.dma_start(wo_sb[:], wo.ap().rearrange("(c p) o -> p c o", p=128))
            nc.sync.dma_start(bq_sb[:], bq.ap())
            nc.sync.dma_start(bk_sb[:], bk.ap())
            nc.sync.dma_start(bv_sb[:], bv.ap())
            make_identity(nc, ident)
            # mask01[j, i] = 1.0 if j <= i else 0.0  (valid causal region in S^T coords)
            make_upper_triangular(nc, mask01, val=1.0, diag=True)
            # ones column for the softmax-denominator trick
            nc.gpsimd.memset(v_sb[:, :, :, HD : HD + 1], 1.0)

            # ---------------- Phase A: qkv projection ----------------
            with (
                tc.tile_pool(name="loadA", bufs=1) as pA,
                tc.tile_pool(name="psumA", bufs=4, space="PSUM") as psA,
            ):
                xT_sb = pA.tile([128, DC, T], F32)
                wq_sb = pA.tile([128, DC, CPC], F32)
                wk_sb = pA.tile([128, DC, CPC], F32)
                wv_sb = pA.tile([128, DC, CPC], F32)
                nc.sync.dma_start(xT_sb[:], xT.ap().rearrange("(o p) t -> p o t", p=128))
                nc.sync.dma_start(wq_sb[:], wq.ap().rearrange("(o p) c -> p o c", p=128))
                nc.sync.dma_start(wk_sb[:], wk.ap().rearrange("(o p) c -> p o c", p=128))
                nc.sync.dma_start(wv_sb[:], wv.ap().rearrange("(o p) c -> p o c", p=128))

                # q^T / k^T : [cols, T] layout
                for dst, w_sb, b_sb in ((qT_sb, wq_sb, bq_sb), (kT_sb, wk_sb, bk_sb)):
                    for hc in range(CC):
                        for tj in range(TW):
                            ps_proj = psA.tile([128, WT], F32, tag="ps_proj")
                            for di in range(DC):
                                nc.tensor.matmul(
                                    ps_proj[:],
                                    w_sb[:, di, hc * 128 : (hc + 1) * 128],
                                    xT_sb[:, di, tj * WT : (tj + 1) * WT],
                                    start=(di == 0),
                                    stop=(di == DC - 1),
                                )
                            nc.vector.tensor_scalar_add(
                                dst[:, hc, tj * WT : (tj + 1) * WT],
                                ps_proj[:],
                                b_sb[:, hc : hc + 1],
                            )

                # v: [T, cols] layout
                for tj in range(TC):
                    ps_v = psA.tile([128, CPC], F32, tag="ps_v", bufs=3)
                    for di in range(DC):
                        nc.tensor.matmul(
                            ps_v[:],
                            xT_sb[:, di, tj * 128 : (tj + 1) * 128],
                            wv_sb[:, di, :],
                            start=(di == 0),
                            stop=(di == DC - 1),
                        )
                    nc.vector.tensor_add(
                        v_sb[:, tj, :, :HD],
                        ps_v[:],
                        bv_sb[:],
                    )

            # ---------------- Phase B: causal attention ----------------
            with (
                tc.tile_pool(name="pB", bufs=2) as pB,
                tc.tile_pool(name="psumB", bufs=4, space="PSUM") as psB,
            ):
                for h in range(HEADS_PER_CORE):
                    hc = h // 2
                    po = (h % 2) * HD
                    for ic in range(TC):
                        pT = pB.tile([128, TC, 128], F32, tag="pT")
                        for jc in range(ic + 1):
                            st = psB.tile([128, 128], F32, tag="st")
                            nc.tensor.matmul(
                                st[:],
                                kT_sb[po : po + HD, hc, jc * 128 : (jc + 1) * 128],
                                qT_sb[po : po + HD, hc, ic * 128 : (ic + 1) * 128],
                                start=True,
                                stop=True,
                            )
                            nc.scalar.activation(
                                pT[:, jc, :], st[:], mybir.ActivationFunctionType.Exp
                            )
                            if jc == ic:
                                nc.vector.tensor_mul(pT[:, jc, :], pT[:, jc, :], mask01[:])
                        pv = psB.tile([128, HD + 1], F32, tag="pv", bufs=2)
                        for jc in range(ic + 1):
                            nc.tensor.matmul(
                                pv[:],
                                pT[:, jc, :],
                                v_sb[:, jc, h, :],
                                start=(jc == 0),
                                stop=(jc == ic),
                            )
                        rl = pB.tile([128, 1], F32, tag="rl", bufs=3)
                        nc.vector.reciprocal(rl[:], pv[:, HD : HD + 1])
                        nc.vector.tensor_scalar_mul(
                            attn_sb[:, ic, h * HD : (h + 1) * HD],
                            pv[:, :HD],
                            rl[:],
                        )

            # ---------------- Phase C+D: transpose + W_o ----------------
            with (
                tc.tile_pool(name="pCD", bufs=1) as pCD,
                tc.tile_pool(name="outp", bufs=3) as outp,
                tc.tile_pool(name="psumCD", bufs=2, space="PSUM") as psCD,
            ):
                attnT_sb = pCD.tile([128, CC, T], F32)
                for ti in range(TC):
                    for dc in range(CC):
                        tp = psCD.tile([128, 128], F32, tag="tp")
                        nc.tensor.transpose(
                            tp[:], attn_sb[:, ti, dc * 128 : (dc + 1) * 128], ident[:]
                        )
                        nc.any.tensor_copy(
                            attnT_sb[:, dc, ti * 128 : (ti + 1) * 128], tp[:]
                        )
                for oc in range(OC):
                    for tj in range(TW):
                        ps_wo = psCD.tile([128, WT], F32, tag="ps_wo")
                        for dc in range(CC):
                            nc.tensor.matmul(
                                ps_wo[:],
                                wo_sb[:, dc, oc * 128 : (oc + 1) * 128],
                                attnT_sb[:, dc, tj * WT : (tj + 1) * WT],
                                start=(dc == 0),
                                stop=(dc == CC - 1),
                            )
                        ot = outp.tile([128, WT], F32, tag="ot")
                        nc.any.tensor_copy(ot[:], ps_wo[:])
                        nc.sync.dma_start(
                            yT.ap()[oc * 128 : (oc + 1) * 128, tj * WT : (tj + 1) * WT],
                            ot[:],
                        )

    nc.compile()
    return nc


def _get_nc():
    if "nc" not in _CACHE:
        _CACHE["nc"] = _build_nc()
    return _CACHE["nc"]


def kernel(x, W_qkv, b_qkv, W_o, b_o, **run_kwargs):
    x = np.asarray(x, dtype=np.float32)
    W_qkv = np.asarray(W_qkv, dtype=np.float32)
    b_qkv = np.asarray(b_qkv, dtype=np.float32)
    W_o = np.asarray(W_o, dtype=np.float32)
    b_o = np.asarray(b_o, dtype=np.float32)

    scale = 1.0 / np.sqrt(np.float32(HD)).astype(np.float32)

    in_maps = []
    for c in range(N_CORES):
        b = c // 2
        g = c % 2
        cs = g * CPC
        q_sl = slice(cs, cs + CPC)
        k_sl = slice(D + cs, D + cs + CPC)
        v_sl = slice(2 * D + cs, 2 * D + cs + CPC)
        in_maps.append(
            {
                "xT": np.ascontiguousarray(x[b].T),
                "wq": np.ascontiguousarray(W_qkv[:, q_sl]) * scale,
                "wk": np.ascontiguousarray(W_qkv[:, k_sl]),
                "wv": np.ascontiguousarray(W_qkv[:, v_sl]),
                "bq": np.ascontiguousarray(
                    (b_qkv[q_sl] * scale).reshape(CC, 128).T
                ),
                "bk": np.ascontiguousarray(b_qkv[k_sl].reshape(CC, 128).T),
                "bv": np.ascontiguousarray(
                    np.broadcast_to(b_qkv[v_sl], (128, CPC))
                ),
                "wo": np.ascontiguousarray(W_o[cs : cs + CPC, :]),
            }
        )

    nc = _get_nc()
    res = run_bass_kernel_spmd(nc, in_maps, core_ids=list(range(N_CORES)), **run_kwargs)
    _CACHE["last_result"] = res

    out = np.empty((B, T, D), dtype=np.float32)
    for b in range(B):
        acc = res.results[2 * b]["yT"] + res.results[2 * b + 1]["yT"]
        out[b] = acc.T + b_o
    return out


# revision 3
# speedup vs baseline: 3.4174x; 3.4174x over previous
import sys

if "/opt/trn_rl_repo" not in sys.path:
    sys.path.insert(0, "/opt/trn_rl_repo")

import numpy as np

import concourse.bass as bass
import concourse.tile as tile
from concourse import bacc, mybir
from concourse.bass_utils import run_bass_kernel_spmd
from concourse.masks import make_identity, make_upper_triangular

F32 = mybir.dt.float32
F32R = mybir.dt.float32r
BF16 = mybir.dt.bfloat16

# Problem shape (hardcoded per contract)
B, T, D = 4, 2048, 768
H, HD = 12, 64
N_CORES = 8
HEADS_PER_CORE = 6          # 12 heads / 2 groups
CPC = HEADS_PER_CORE * HD   # 384 qkv columns per core
TC = T // 128               # 16 token tiles of 128
DC = D // 128               # 6 chunks of the model dim
CC = CPC // 128             # 3 chunks of this core's head cols
OC = D // 128               # 6 output-col chunks
WT = 512                    # wide tile for big matmuls
TW = T // WT                # 4 wide token tiles
GRP = 8                     # score j-tiles staged per PSUM exp batch

_CACHE = {}


def _build_nc():
    nc = bacc.Bacc("TRN2", target_bir_lowering=False, debug=False)

    xT = nc.dram_tensor("xT", [D, T], F32R, kind="ExternalInput")
    wq = nc.dram_tensor("wq", [D, CPC], F32R, kind="ExternalInput")
    wk = nc.dram_tensor("wk", [D, CPC], F32R, kind="ExternalInput")
    wv = nc.dram_tensor("wv", [D, CPC], F32R, kind="ExternalInput")
    bq = nc.dram_tensor("bq", [128, CC], F32, kind="ExternalInput")
    bk = nc.dram_tensor("bk", [128, CC], F32, kind="ExternalInput")
    bv = nc.dram_tensor("bv", [128, CPC], F32, kind="ExternalInput")
    wo = nc.dram_tensor("wo", [CPC, D], F32R, kind="ExternalInput")
    yT = nc.dram_tensor("yT", [D, T], F32, kind="ExternalOutput")

    with tile.TileContext(nc) as tc:
        with tc.tile_pool(name="persist", bufs=1) as pp:
            qT_sb = pp.tile([128, CC, T], BF16)     # q^T, head cols on partitions
            kT_sb = pp.tile([128, CC, T], BF16)
            v_sb = pp.tile([128, TC, HEADS_PER_CORE, HD + 1], BF16)  # v | ones
            attn_sb = pp.tile([128, TC, CPC], F32)   # attention out [T, cols]
            attnT_sb = pp.tile([128, CC, T], F32R)   # transposed for W_o
            wo_sb = pp.tile([128, CC, D], F32R)
            bq_sb = pp.tile([128, CC], F32)
            bk_sb = pp.tile([128, CC], F32)
            bv_sb = pp.tile([128, CPC], F32)
            ident = pp.tile([128, 128], F32)
            maskf = pp.tile([128, 128], F32)
            mask01 = pp.tile([128, 128], BF16)

            nc.sync.dma_start(wo_sb[:], wo.ap().rearrange("(c p) o -> p c o", p=128))
            nc.sync.dma_start(bq_sb[:], bq.ap())
            nc.sync.dma_start(bk_sb[:], bk.ap())
            nc.sync.dma_start(bv_sb[:], bv.ap())
            make_identity(nc, ident)
            # mask01[j, i] = 1.0 if j <= i else 0.0 (valid causal region, S^T coords)
            make_upper_triangular(nc, maskf, val=1.0, diag=True)
            nc.vector.tensor_copy(mask01[:], maskf[:])
            nc.gpsimd.memset(v_sb[:, :, :, HD : HD + 1], 1.0)

            # ---------------- Phase A: qkv projection ----------------
            with (
                tc.tile_pool(name="loadA", bufs=1) as pA,
                tc.tile_pool(name="psumA", bufs=3, space="PSUM") as psA,
            ):
                xT_sb = pA.tile([128, DC, T], F32R)
                wq_sb = pA.tile([128, DC, CPC], F32R)
                wk_sb = pA.tile([128, DC, CPC], F32R)
                wv_sb = pA.tile([128, DC, CPC], F32R)
                nc.sync.dma_start(xT_sb[:], xT.ap().rearrange("(o p) t -> p o t", p=128))
                nc.sync.dma_start(wq_sb[:], wq.ap().rearrange("(o p) c -> p o c", p=128))
                nc.sync.dma_start(wk_sb[:], wk.ap().rearrange("(o p) c -> p o c", p=128))
                nc.sync.dma_start(wv_sb[:], wv.ap().rearrange("(o p) c -> p o c", p=128))

                # v first: phase B consumes v for ic=0 immediately
                for tj in range(TC):
                    ps_v = psA.tile([128, CPC], F32, tag="ps_v")
                    for di in range(DC):
                        nc.tensor.matmul(
                            ps_v[:],
                            xT_sb[:, di, tj * 128 : (tj + 1) * 128],
                            wv_sb[:, di, :],
                            start=(di == 0),
                            stop=(di == DC - 1),
                        )
                    nc.vector.tensor_add(v_sb[:, tj, :, :HD], ps_v[:], bv_sb[:])

                for dst, w_sb, b_sb in ((qT_sb, wq_sb, bq_sb), (kT_sb, wk_sb, bk_sb)):
                    for hc in range(CC):
                        for tj in range(TW):
                            ps_proj = psA.tile([128, WT], F32, tag="ps_proj")
                            for di in range(DC):
                                nc.tensor.matmul(
                                    ps_proj[:],
                                    w_sb[:, di, hc * 128 : (hc + 1) * 128],
                                    xT_sb[:, di, tj * WT : (tj + 1) * WT],
                                    start=(di == 0),
                                    stop=(di == DC - 1),
                                )
                            nc.vector.tensor_scalar_add(
                                dst[:, hc, tj * WT : (tj + 1) * WT],
                                ps_proj[:],
                                b_sb[:, hc : hc + 1],
                            )

            # ---------- Phase B: causal attention (+C/D interleaved) ----------
            with (
                tc.tile_pool(name="pB", bufs=2) as pB,
                tc.tile_pool(name="outp", bufs=3) as outp,
                tc.tile_pool(name="psumB", bufs=2, space="PSUM") as psB,
                tc.tile_pool(name="psumCD", bufs=1, space="PSUM") as psCD,
            ):
                for ic in range(TC):
                    n_j = ic + 1
                    for h in range(HEADS_PER_CORE):
                        hc = h // 2
                        po = (h % 2) * HD
                        pT = pB.tile([128, TC, 128], BF16, tag="pT")
                        for g in range(0, n_j, GRP):
                            ge = min(n_j, g + GRP)
                            st = psB.tile([128, GRP * 128], F32, tag="st")
                            st3 = st.rearrange("p (a b) -> p a b", b=128)
                            for jc in range(g, ge):
                                nc.tensor.matmul(
                                    st3[:, jc - g, :],
                                    kT_sb[po : po + HD, hc, jc * 128 : (jc + 1) * 128],
                                    qT_sb[po : po + HD, hc, ic * 128 : (ic + 1) * 128],
                                    start=True,
                                    stop=True,
                                )
                            nc.scalar.activation(
                                pT[:, g:ge, :],
                                st3[:, : ge - g, :],
                                mybir.ActivationFunctionType.Exp,
                            )
                        nc.vector.tensor_mul(pT[:, ic, :], pT[:, ic, :], mask01[:])
                        pv = psB.tile([128, HD + 1], F32, tag="pv")
                        for jc in range(n_j):
                            nc.tensor.matmul(
                                pv[:],
                                pT[:, jc, :],
                                v_sb[:, jc, h, :],
                                start=(jc == 0),
                                stop=(jc == n_j - 1),
                            )
                        rl = pB.tile([128, 1], F32, tag="rl", bufs=3)
                        nc.vector.reciprocal(rl[:], pv[:, HD : HD + 1])
                        nc.vector.tensor_scalar_mul(
                            attn_sb[:, ic, h * HD : (h + 1) * HD], pv[:, :HD], rl[:]
                        )

                    # phase C: transpose this token tile for W_o
                    for dc in range(CC):
                        tp = psCD.tile([128, 128], F32, tag="tp")
                        nc.tensor.transpose(
                            tp[:], attn_sb[:, ic, dc * 128 : (dc + 1) * 128], ident[:]
                        )
                        nc.vector.tensor_copy(
                            attnT_sb[:, dc, ic * 128 : (ic + 1) * 128], tp[:]
                        )

                    # phase D: W_o for each completed 512-wide token block
                    if ic % 4 == 3:
                        tj = ic // 4
                        for oc in range(OC):
                            ps_wo = psCD.tile([128, WT], F32, tag="ps_wo")
                            for dc in range(CC):
                                nc.tensor.matmul(
                                    ps_wo[:],
                                    wo_sb[:, dc, oc * 128 : (oc + 1) * 128],
                                    attnT_sb[:, dc, tj * WT : (tj + 1) * WT],
                                    start=(dc == 0),
                                    stop=(dc == CC - 1),
                                )
                            ot = outp.tile([128, WT], F32, tag="ot")
                            nc.scalar.copy(ot[:], ps_wo[:])
                            nc.sync.dma_start(
                                yT.ap()[
                                    oc * 128 : (oc + 1) * 128, tj * WT : (tj + 1) * WT
                                ],
                                ot[:],
                            )

    nc.compile()
    return nc


def _get_nc():
    if "nc" not in _CACHE:
        _CACHE["nc"] = _build_nc()
    return _CACHE["nc"]


def kernel(x, W_qkv, b_qkv, W_o, b_o, **run_kwargs):
    x = np.asarray(x, dtype=np.float32)
    W_qkv = np.asarray(W_qkv, dtype=np.float32)
    b_qkv = np.asarray(b_qkv, dtype=np.float32)
    W_o = np.asarray(W_o, dtype=np.float32)
    b_o = np.asarray(b_o, dtype=np.float32)

    scale = np.float32(1.0) / np.sqrt(np.float32(HD)).astype(np.float32)

    in_maps = []
    for c in range(N_CORES):
        b = c // 2
        g = c % 2
        cs = g * CPC
        q_sl = slice(cs, cs + CPC)
        k_sl = slice(D + cs, D + cs + CPC)
        v_sl = slice(2 * D + cs, 2 * D + cs + CPC)
        in_maps.append(
            {
                "xT": np.ascontiguousarray(x[b].T),
                "wq": np.ascontiguousarray(W_qkv[:, q_sl]) * scale,
                "wk": np.ascontiguousarray(W_qkv[:, k_sl]),
                "wv": np.ascontiguousarray(W_qkv[:, v_sl]),
                "bq": np.ascontiguousarray((b_qkv[q_sl] * scale).reshape(CC, 128).T),
                "bk": np.ascontiguousarray(b_qkv[k_sl].reshape(CC, 128).T),
                "bv": np.ascontiguousarray(np.broadcast_to(b_qkv[v_sl], (128, CPC))),
                "wo": np.ascontiguousarray(W_o[cs : cs + CPC, :]),
            }
        )

    nc = _get_nc()
    res = run_bass_kernel_spmd(nc, in_maps, core_ids=list(range(N_CORES)), **run_kwargs)
    _CACHE["last_result"] = res

    out = np.empty((B, T, D), dtype=np.float32)
    for b in range(B):
        acc = res.results[2 * b]["yT"] + res.results[2 * b + 1]["yT"]
        out[b] = acc.T + b_o
    return out


# revision 5
# speedup vs baseline: 3.7046x; 1.0840x over previous
import sys

if "/opt/trn_rl_repo" not in sys.path:
    sys.path.insert(0, "/opt/trn_rl_repo")

import numpy as np

import concourse.bass as bass
import concourse.tile as tile
from concourse import bacc, mybir
from concourse.bass_utils import run_bass_kernel_spmd
from concourse.masks import make_identity, make_upper_triangular

F32 = mybir.dt.float32
F32R = mybir.dt.float32r
BF16 = mybir.dt.bfloat16

# Problem shape (hardcoded per contract)
B, T, D = 4, 2048, 768
H, HD = 12, 64
N_CORES = 8
HEADS_PER_CORE = 6          # 12 heads / 2 groups
CPC = HEADS_PER_CORE * HD   # 384 qkv columns per core
TC = T // 128               # 16 token tiles of 128
DC = D // 128               # 6 chunks of the model dim
CC = CPC // 128             # 3 chunks of this core's head cols
OC = D // 128               # 6 output-col chunks
WT = 512                    # wide tile for big matmuls
TW = T // WT                # 4 wide token tiles
GRP = 8                     # score j-tiles staged per PSUM exp batch

_CACHE = {}


def _build_nc():
    nc = bacc.Bacc("TRN2", target_bir_lowering=False, debug=False)

    xT = nc.dram_tensor("xT", [D, T], F32R, kind="ExternalInput")
    wq = nc.dram_tensor("wq", [D, CPC], F32R, kind="ExternalInput")
    wk = nc.dram_tensor("wk", [D, CPC], F32R, kind="ExternalInput")
    wv = nc.dram_tensor("wv", [D, CPC], F32R, kind="ExternalInput")
    bq = nc.dram_tensor("bq", [128, CC], F32, kind="ExternalInput")
    bk = nc.dram_tensor("bk", [128, CC], F32, kind="ExternalInput")
    bv = nc.dram_tensor("bv", [128, CPC], F32, kind="ExternalInput")
    wo = nc.dram_tensor("wo", [CPC, D], F32R, kind="ExternalInput")
    yT = nc.dram_tensor("yT", [D, T], F32, kind="ExternalOutput")

    with tile.TileContext(nc) as tc:
        with tc.tile_pool(name="persist", bufs=1) as pp:
            qT_sb = pp.tile([128, CC, T], BF16)     # q^T, head cols on partitions
            kT_sb = pp.tile([128, CC, T], BF16)
            v_sb = pp.tile([128, TC, HEADS_PER_CORE, HD + 1], BF16)  # v | ones
            attn_sb = pp.tile([128, TC, CPC], F32)   # attention out [T, cols]
            attnT_sb = pp.tile([128, CC, T], F32R)   # transposed for W_o
            wo_sb = pp.tile([128, CC, D], F32R)
            bq_sb = pp.tile([128, CC], F32)
            bk_sb = pp.tile([128, CC], F32)
            bv_sb = pp.tile([128, CPC], F32)
            ident = pp.tile([128, 128], F32)
            maskf = pp.tile([128, 128], F32)
            mask01 = pp.tile([128, 128], BF16)

            nc.sync.dma_start(wo_sb[:], wo.ap().rearrange("(c p) o -> p c o", p=128))
            nc.sync.dma_start(bq_sb[:], bq.ap())
            nc.sync.dma_start(bk_sb[:], bk.ap())
            nc.sync.dma_start(bv_sb[:], bv.ap())
            make_identity(nc, ident)
            # mask01[j, i] = 1.0 if j <= i else 0.0 (valid causal region, S^T coords)
            make_upper_triangular(nc, maskf, val=1.0, diag=True)
            nc.vector.tensor_copy(mask01[:], maskf[:])
            nc.gpsimd.memset(v_sb[:, :, :, HD : HD + 1], 1.0)

            # ---------------- Phase A: qkv projection ----------------
            with (
                tc.tile_pool(name="loadA", bufs=1) as pA,
                tc.tile_pool(name="psumA", bufs=3, space="PSUM") as psA,
            ):
                xT_sb = pA.tile([128, DC, T], F32R)
                wq_sb = pA.tile([128, DC, CPC], F32R)
                wk_sb = pA.tile([128, DC, CPC], F32R)
                wv_sb = pA.tile([128, DC, CPC], F32R)
                nc.sync.dma_start(xT_sb[:], xT.ap().rearrange("(o p) t -> p o t", p=128))
                nc.sync.dma_start(wq_sb[:], wq.ap().rearrange("(o p) c -> p o c", p=128))
                nc.sync.dma_start(wk_sb[:], wk.ap().rearrange("(o p) c -> p o c", p=128))
                nc.sync.dma_start(wv_sb[:], wv.ap().rearrange("(o p) c -> p o c", p=128))

                # v first: phase B consumes v for ic=0 immediately
                for tj in range(TC):
                    ps_v = psA.tile([128, CPC], F32, tag="ps_v")
                    for di in range(DC):
                        nc.tensor.matmul(
                            ps_v[:],
                            xT_sb[:, di, tj * 128 : (tj + 1) * 128],
                            wv_sb[:, di, :],
                            start=(di == 0),
                            stop=(di == DC - 1),
                        )
                    nc.vector.tensor_add(v_sb[:, tj, :, :HD], ps_v[:], bv_sb[:])

                for dst, w_sb, b_sb in ((qT_sb, wq_sb, bq_sb), (kT_sb, wk_sb, bk_sb)):
                    for hc in range(CC):
                        for tj in range(TW):
                            ps_proj = psA.tile([128, WT], F32, tag="ps_proj")
                            for di in range(DC):
                                nc.tensor.matmul(
                                    ps_proj[:],
                                    w_sb[:, di, hc * 128 : (hc + 1) * 128],
                                    xT_sb[:, di, tj * WT : (tj + 1) * WT],
                                    start=(di == 0),
                                    stop=(di == DC - 1),
                                )
                            nc.vector.tensor_scalar_add(
                                dst[:, hc, tj * WT : (tj + 1) * WT],
                                ps_proj[:],
                                b_sb[:, hc : hc + 1],
                            )

            # ---------- Phase B: causal attention (+C/D interleaved) ----------
            with (
                tc.tile_pool(name="pB", bufs=2) as pB,
                tc.tile_pool(name="outp", bufs=3) as outp,
                tc.tile_pool(name="psumB", bufs=2, space="PSUM") as psB,
                tc.tile_pool(name="psumCD", bufs=1, space="PSUM") as psCD,
            ):
                for ic in reversed(range(TC)):
                    n_j = ic + 1
                    pTs = []
                    for h in range(HEADS_PER_CORE):
                        hc = h // 2
                        po = (h % 2) * HD
                        pT = pB.tile(
                            [128, TC, 128], BF16, tag="pT", bufs=HEADS_PER_CORE + 1
                        )
                        pTs.append(pT)
                        for g in range(0, n_j, GRP):
                            ge = min(n_j, g + GRP)
                            st = psB.tile([128, GRP * 128], F32, tag="st")
                            st3 = st.rearrange("p (a b) -> p a b", b=128)
                            for jc in range(g, ge):
                                nc.tensor.matmul(
                                    st3[:, jc - g, :],
                                    kT_sb[po : po + HD, hc, jc * 128 : (jc + 1) * 128],
                                    qT_sb[po : po + HD, hc, ic * 128 : (ic + 1) * 128],
                                    start=True,
                                    stop=True,
                                )
                            nc.scalar.activation(
                                pT[:, g:ge, :],
                                st3[:, : ge - g, :],
                                mybir.ActivationFunctionType.Exp,
                            )
                        nc.vector.tensor_mul(pT[:, ic, :], pT[:, ic, :], mask01[:])
                    for h in range(HEADS_PER_CORE):
                        pT = pTs[h]
                        pv = psB.tile([128, HD + 1], F32, tag="pv")
                        for jc in range(n_j):
                            nc.tensor.matmul(
                                pv[:],
                                pT[:, jc, :],
                                v_sb[:, jc, h, :],
                                start=(jc == 0),
                                stop=(jc == n_j - 1),
                            )
                        rl = pB.tile([128, 1], F32, tag="rl", bufs=3)
                        nc.vector.reciprocal(rl[:], pv[:, HD : HD + 1])
                        nc.vector.tensor_scalar_mul(
                            attn_sb[:, ic, h * HD : (h + 1) * HD], pv[:, :HD], rl[:]
                        )

                    # phase C: transpose this token tile for W_o
                    for dc in range(CC):
                        tp = psCD.tile([128, 128], F32, tag="tp")
                        nc.tensor.transpose(
                            tp[:], attn_sb[:, ic, dc * 128 : (dc + 1) * 128], ident[:]
                        )
                        nc.vector.tensor_copy(
                            attnT_sb[:, dc, ic * 128 : (ic + 1) * 128], tp[:]
                        )

                    # phase D: W_o for each completed 512-wide token block
                    # (descending ic: block tj is complete when ic == 4*tj)
                    if ic % 4 == 0:
                        tj = ic // 4
                        for oc in range(OC):
                            ps_wo = psCD.tile([128, WT], F32, tag="ps_wo")
                            for dc in range(CC):
                                nc.tensor.matmul(
                                    ps_wo[:],
                                    wo_sb[:, dc, oc * 128 : (oc + 1) * 128],
                                    attnT_sb[:, dc, tj * WT : (tj + 1) * WT],
                                    start=(dc == 0),
                                    stop=(dc == CC - 1),
                                )
                            ot = outp.tile([128, WT], F32, tag="ot")
                            nc.scalar.copy(ot[:], ps_wo[:])
                            nc.sync.dma_start(
                                yT.ap()[
                                    oc * 128 : (oc + 1) * 128, tj * WT : (tj + 1) * WT
                                ],
                                ot[:],
                            )

    nc.compile()
    return nc


def _get_nc():
    if "nc" not in _CACHE:
        _CACHE["nc"] = _build_nc()
    return _CACHE["nc"]


def kernel(x, W_qkv, b_qkv, W_o, b_o, **run_kwargs):
    x = np.asarray(x, dtype=np.float32)
    W_qkv = np.asarray(W_qkv, dtype=np.float32)
    b_qkv = np.asarray(b_qkv, dtype=np.float32)
    W_o = np.asarray(W_o, dtype=np.float32)
    b_o = np.asarray(b_o, dtype=np.float32)

    scale = np.float32(1.0) / np.sqrt(np.float32(HD)).astype(np.float32)

    in_maps = []
    for c in range(N_CORES):
        b = c // 2
        g = c % 2
        cs = g * CPC
        q_sl = slice(cs, cs + CPC)
        k_sl = slice(D + cs, D + cs + CPC)
        v_sl = slice(2 * D + cs, 2 * D + cs + CPC)
        in_maps.append(
            {
                "xT": np.ascontiguousarray(x[b].T),
                "wq": np.ascontiguousarray(W_qkv[:, q_sl]) * scale,
                "wk": np.ascontiguousarray(W_qkv[:, k_sl]),
                "wv": np.ascontiguousarray(W_qkv[:, v_sl]),
                "bq": np.ascontiguousarray((b_qkv[q_sl] * scale).reshape(CC, 128).T),
                "bk": np.ascontiguousarray(b_qkv[k_sl].reshape(CC, 128).T),
                "bv": np.ascontiguousarray(np.broadcast_to(b_qkv[v_sl], (128, CPC))),
                "wo": np.ascontiguousarray(W_o[cs : cs + CPC, :]),
            }
        )

    nc = _get_nc()
    res = run_bass_kernel_spmd(nc, in_maps, core_ids=list(range(N_CORES)), **run_kwargs)
    _CACHE["last_result"] = res

    out = np.empty((B, T, D), dtype=np.float32)
    for b in range(B):
        acc = res.results[2 * b]["yT"] + res.results[2 * b + 1]["yT"]
        out[b] = acc.T + b_o
    return out
